# revision 10
# baseline (speedup 1.0000x reference)
"""Trainium2 Bass kernel for BCNLayer (3x3 per-position-weighted spatial
shift conv over a 128x128 grid + sigmoid).

y[yo,xo,b] = sigmoid( sum_{dy,dx in {-1,0,1}} w[dy+1,dx+1,(yo-dy)*128+(xo-dx)]
                      * x[(yo-dy)*128+(xo-dx), b] )   (zero outside the grid)

Formulation: for each output row yo, y_row[yo] = sigmoid( sum_{yi in
{yo-1,yo,yo+1}} T[dy,yi].T @ x_row[yi] ) where T[dy,yi] is a 128x128
tridiagonal matrix holding the three dx weight vectors of input row yi on
its diagonals (dy = yo-yi).  The whole pipeline runs in fp16 (x, w and y
all round-trip through fp16, which costs ~1e-3 absolute on a sigmoid
output -- far inside the 2e-2 gate):

  * x chunks load through SWDGE DMAs that cast f32 -> fp16 inline, which
    halves the load bytes the DMA engines move;
  * T matrices are built fp16 on-chip from an fp16 SBUF weight image
    with one-hot diagonal masks (one tensor_tensor multiply on Pool +
    two DVE predicated copies per dy; CopyPredicated is DVE-only on
    TRN2 and cannot write f32r, but writes fp16 natively, so the f32r
    rounding pass of the f32 variant disappears);
  * matmuls are fp16 x fp16 -> f32 PSUM (1 cycle/row moving);
  * output rows accumulate into a 4-bank PSUM tile so one sigmoid
    activation covers 4 rows, stored fp16 by the idle SP sequencer.

A 130-wide T buffer with the matmul reading cols 1:129 makes the
x-boundary masking fall out of the padding columns.

Sharding: data-parallel over batch, 4096/8 = 512 columns per core.
"""

import numpy as np

H = 128
W = 128
HW = H * W
B = 4096
NCORES = 8
BC = B // NCORES  # 512 batch columns per core
G = 8  # yi rows per weight-group tile
R = 4  # y rows per PSUM tile / sigmoid / store DMA (512 KiB fp16)
LR = 16  # x rows per load DMA (16 * 128 part * 512 * 2B = 2 MiB fp16)

_CACHE = {}


def _make_tile_context_cls():
    import concourse.tile as tile
    import bass_rust

    class SplitDrainTileContext(tile.TileContext):
        """The walrus build in this container accepts at most one sem-wait
        per instruction; Tile freely emits several (e.g. a matmul waiting
        on both operand DMA lanes).  Split the extras onto single-wait
        nops emitted just before the instruction on the same engine."""

        def _add_instruction(self, inst):
            from concourse import mybir as _mybir

            si = inst.sync_info
            if si is not None and si.on_wait and len(si.on_wait) > 1:
                waits = list(si.on_wait)
                si.on_wait = [waits[-1]]
                for w in waits[:-1]:
                    nop = _mybir.InstNoOp(
                        name=self.nc.get_next_instruction_name(),
                        ins=[],
                        outs=[],
                    )
                    nop.engine = inst.engine
                    nop.sync_info = _mybir.SyncInfo(on_wait=[w], on_update=[])
                    super()._add_instruction(nop)
            super()._add_instruction(inst)

        def _drain_and_barrier(self, tick_clock, wait_clock):
            collector = self.nc.sync.nop(nofuse=True, hint="tail_waits")
            wait_clock.add_sem_waits(
                collector.ins,
                bass_rust.ScopedClock({None: tick_clock.global_clock}),
            )
            si = collector.ins.sync_info
            waits = list(si.on_wait) if si is not None and si.on_wait else []
            if len(waits) > 1:
                si.on_wait = [waits[0]]
                from concourse import mybir as _mybir

                for w in waits[1:]:
                    n = self.nc.sync.nop(nofuse=True, hint="tail_waits")
                    n.ins.sync_info = _mybir.SyncInfo(on_wait=[w], on_update=[])
            self.nc.sync.drain()
            self.nc.all_engine_barrier()
            assert self.sems is not None
            popped = self.nc._tile_sem_poison_stack.pop()
            assert popped is self._sem_poison
            self.nc.clear_and_free_semaphores(
                list(self.sems.allocated().values())
            )
            self.nc.all_engine_barrier()

    return SplitDrainTileContext


def _build_nc(repeat=1):
    import concourse.bass as bass
    import concourse.tile as tile
    import concourse.mybir as mybir
    from concourse.ap import AP

    tile_context_cls = _make_tile_context_cls()
    f32 = mybir.dt.float32
    f16 = mybir.dt.float16
    nc = bass.Bass("TRN2", target_bir_lowering=False, debug=False)
    x = nc.dram_tensor("x", [HW, BC], f32, kind="ExternalInput")
    # wsb[xi, (i, yi, j)] = w[i, j, yi*128+xi]: the per-partition SBUF
    # image of the weights, prepared host-side; SWDGE casts it to fp16
    wsb_d = nc.dram_tensor("wsb", [128, 1152], f32, kind="ExternalInput")
    # y stored fp16 (sigmoid output in [0,1]: adds <= ~2.4e-4 abs error)
    # and upcast to f32 on the host -- halves the output DMA traffic
    y = nc.dram_tensor("y", [HW, BC], f16, kind="ExternalOutput")

    NCH = H // LR  # 16 x row-chunks
    NGR = H // G  # 16 weight groups
    TW = 130  # T used width: col c = xi + j, lhsT reads cols 1:129
    TWS = 131  # T stored stride (!=TW so strided APs never dim-merge)

    with tile_context_cls(nc) as tc:
        with (
            tc.tile_pool(name="cn", bufs=1) as cpool,
            tc.tile_pool(name="xp", bufs=4) as xpool,
            tc.tile_pool(name="tp", bufs=7) as tpool,
            tc.tile_pool(name="op", bufs=4) as opool,
            tc.tile_pool(name="ps", bufs=2, space="PSUM") as ppool,
        ):
            # one-time: weight image (cast f32 -> fp16 in the DMA),
            # one-hot diagonal masks
            wsb = cpool.tile([128, 1152], f16)
            nc.gpsimd.dma_start(out=wsb[:], in_=wsb_d.ap())
            i16 = mybir.dt.int16
            ones = cpool.tile([128, TW], i16)
            nc.gpsimd.memset(ones[:], 1)
            onesf = cpool.tile([128, TW], f16)
            nc.gpsimd.memset(onesf[:], 1.0)
            masks = cpool.tile([128, 3, TW], i16)
            for j in range(1, 3):
                # D_j[xi, c] = 1 where c - xi - j == 0
                nc.gpsimd.affine_select(
                    masks[:, j, :], ones[:],
                    pattern=[[1, TW]], base=-j, channel_multiplier=-1,
                    compare_op=mybir.AluOpType.is_equal, fill=0,
                )
            # fp16 one-hot for j=0 (used multiplicatively: zero-fills too)
            mask0f = cpool.tile([128, TW], f16)
            nc.gpsimd.affine_select(
                mask0f[:], onesf[:],
                pattern=[[1, TW]], base=0, channel_multiplier=-1,
                compare_op=mybir.AluOpType.is_equal, fill=0.0,
            )

            xt = {}
            tt = {}

            def load_chunk(c):  # noqa: closure rebound per repeat
                if c in xt or c >= NCH:
                    return
                t = xpool.tile([128, LR, BC], f16, tag="xchunk")
                # x rows [c*LR*128, (c+1)*LR*128): the SWDGE (gpsimd) DMA
                # casts f32 -> fp16 inline, so the DMA engines only move
                # half the bytes; 2 MiB per DMA keeps the Pool-side
                # descriptor generation cheap
                src = AP(
                    x.ap().tensor,
                    c * LR * 128 * BC,
                    [[BC, 128], [128 * BC, LR], [1, BC]],
                )
                nc.gpsimd.dma_start(out=t[:], in_=src)
                xt[c] = t

            def load_group(g):
                if g in tt or g >= NGR:
                    return
                t = tpool.tile([128, 3, G, TWS], f16, tag="T")
                ta = t[:]
                wv = wsb[:]
                for i in range(3):
                    out_i = AP(ta.tensor, ta.offset + i * G * TWS,
                               [[3 * G * TWS, 128], [TWS, G], [1, TW]])

                    def wb(j):
                        return AP(wv.tensor,
                                  wv.offset + i * 384 + g * G * 3 + j,
                                  [[1152, 128], [3, G], [0, TW]])

                    # j=0 as a multiply by the fp16 one-hot: zero-fills the
                    # whole block and places the j=0 diagonal in one pass
                    # (on gpsimd to keep DVE free for the predicated passes)
                    m0 = mask0f[:]
                    m0b = AP(m0.tensor, m0.offset,
                             [[TW, 128], [0, G], [1, TW]])
                    nc.gpsimd.tensor_tensor(
                        out_i, m0b, wb(0), mybir.AluOpType.mult
                    )
                    for j in range(1, 3):
                        ma = masks[:, j, :]
                        mb = AP(ma.tensor, ma.offset,
                                [[3 * TW, 128], [0, G], [1, TW]])
                        nc.vector.copy_predicated(out_i, mb, wb(j))
                tt[g] = t

            rep_range = range(repeat)
            for _rep in rep_range:
              if _rep:
                xt.clear()
                tt.clear()
              # prime the pipeline: interleave the first chunk with the
              # first T groups so the opening matmul isn't queued behind
              # all the SWDGE descriptor-generation bursts on Pool
              load_chunk(0)
              load_group(0)
              load_group(1)
              load_chunk(1)
              load_group(2)
              load_chunk(2)
              for _g in range(3, 7):
                  load_group(_g)

              pt = None
              ystage = None
              for yo in range(H):
                  # prefetch beyond what this row touches
                  load_chunk((yo + 1) // LR + 1)
                  load_group((yo + 1) // G + 2)
                  load_group((yo + 1) // G + 3)

                  if yo % R == 0:
                      # 4 banks: rows yo..yo+3 accumulate side by side so
                      # one sigmoid covers all 4
                      pt = ppool.tile([128, R, BC], f32, tag="psum")
                      ystage = opool.tile([128, R, BC], f16, tag="yst")
                  yis = [yi for yi in (yo - 1, yo, yo + 1) if 0 <= yi < H]
                  for k, yi in enumerate(yis):
                      i_dy = yo - yi + 1
                      lhsT = tt[yi // G][:, i_dy, yi % G, 1 : 1 + 128]  # [128, 128]
                      rhs = xt[yi // LR][:, yi % LR, :]
                      nc.tensor.matmul(
                          pt[:, yo % R, :],
                          lhsT,
                          rhs,
                          start=(k == 0),
                          stop=(k == len(yis) - 1),
                      )

                  if yo % R == R - 1:
                      nc.scalar.activation(
                          ystage[:],
                          pt[:],
                          mybir.ActivationFunctionType.Sigmoid,
                      )
                      c = yo // R
                      dst = AP(
                          y.ap().tensor,
                          c * R * 128 * BC,
                          [[BC, 128], [128 * BC, R], [1, BC]],
                      )
                      # stores issue from SP: its HWDGE ring is otherwise
                      # idle, keeping ACT free for the sigmoids
                      nc.sync.dma_start(out=dst, in_=ystage[:])
    return nc


def get_nc():
    if "nc" not in _CACHE:
        _CACHE["nc"] = _build_nc()
    return _CACHE["nc"]


def kernel(x: np.ndarray, w: np.ndarray) -> np.ndarray:
    import time as _time

    from concourse.bass_utils import run_bass_kernel_spmd

    x = np.ascontiguousarray(x, dtype=np.float32)
    wsb = np.ascontiguousarray(
        np.asarray(w, dtype=np.float32)
        .reshape(3, 3, H, W)
        .transpose(3, 0, 2, 1)
        .reshape(128, 1152)
    )
    nc = get_nc()
    in_maps = [
        {"x": x[:, i * BC : (i + 1) * BC], "wsb": wsb} for i in range(NCORES)
    ]
    # The compile hook / remote execution path occasionally fails
    # transiently (observed: a flaky walrus invocation and a recoverable
    # NRT exec error); retry a few times before giving up.
    last_exc = None
    for attempt in range(4):
        try:
            res = run_bass_kernel_spmd(
                nc, in_maps, list(range(NCORES))
            ).results
            break
        except Exception as exc:  # noqa: BLE001
            last_exc = exc
            _time.sleep(2.0 * (attempt + 1))
    else:
        raise last_exc
    return np.ascontiguousarray(
        np.concatenate([res[i]["y"] for i in range(NCORES)], axis=1),
        dtype=np.float32,
    )


# revision 14
# speedup vs baseline: 1.3790x; 1.3790x over previous
"""Trainium2 Bass kernel for BCNLayer (3x3 per-position-weighted spatial
shift conv over a 128x128 grid + sigmoid).

y[yo,xo,b] = sigmoid( sum_{dy,dx in {-1,0,1}} w[dy+1,dx+1,(yo-dy)*128+(xo-dx)]
                      * x[(yo-dy)*128+(xo-dx), b] )   (zero outside the grid)

Formulation: for each output row yo, y_row[yo] = sigmoid( sum_{yi in
{yo-1,yo,yo+1}} T[dy,yi].T @ x_row[yi] ) where T[dy,yi] is a 128x128
tridiagonal matrix holding the three dx weight vectors of input row yi on
its diagonals (dy = yo-yi).  The whole pipeline runs in fp16 (x, w and y
all round-trip through fp16, which costs ~1e-3 absolute on a sigmoid
output -- far inside the 2e-2 gate):

  * x chunks load through SWDGE DMAs that cast f32 -> fp16 inline, which
    halves the load bytes the DMA engines move;
  * T matrices are built fp16 on-chip from an fp16 SBUF weight image
    with one-hot diagonal masks (one tensor_tensor multiply on Pool +
    two DVE predicated copies per dy; CopyPredicated is DVE-only on
    TRN2 and cannot write f32r, but writes fp16 natively, so the f32r
    rounding pass of the f32 variant disappears);
  * matmuls are fp16 x fp16 -> f32 PSUM (1 cycle/row moving);
  * output rows accumulate into a 4-bank PSUM tile so one sigmoid
    activation covers 4 rows, stored fp16 by the idle SP sequencer.

A 130-wide T buffer with the matmul reading cols 1:129 makes the
x-boundary masking fall out of the padding columns.

Sharding: data-parallel over batch, 4096/8 = 512 columns per core.
"""

import numpy as np

H = 128
W = 128
HW = H * W
B = 4096
NCORES = 8
BC = B // NCORES  # 512 batch columns per core
G = 8  # yi rows per weight-group tile
R = 4  # y rows per PSUM tile / sigmoid / store DMA (512 KiB fp16)
LR = 8  # x rows per load DMA (8 * 128 part * 512 * 2B = 1 MiB fp16;
#   1024 descriptors -- exactly the SWDGE queue carveout)

_CACHE = {}


def _make_tile_context_cls():
    import concourse.tile as tile
    import bass_rust

    class SplitDrainTileContext(tile.TileContext):
        """The walrus build in this container accepts at most one sem-wait
        per instruction; Tile freely emits several (e.g. a matmul waiting
        on both operand DMA lanes).  Split the extras onto single-wait
        nops emitted just before the instruction on the same engine."""

        def _add_instruction(self, inst):
            from concourse import mybir as _mybir

            si = inst.sync_info
            if si is not None and si.on_wait and len(si.on_wait) > 1:
                waits = list(si.on_wait)
                si.on_wait = [waits[-1]]
                for w in waits[:-1]:
                    nop = _mybir.InstNoOp(
                        name=self.nc.get_next_instruction_name(),
                        ins=[],
                        outs=[],
                    )
                    nop.engine = inst.engine
                    nop.sync_info = _mybir.SyncInfo(on_wait=[w], on_update=[])
                    super()._add_instruction(nop)
            super()._add_instruction(inst)

        def _drain_and_barrier(self, tick_clock, wait_clock):
            collector = self.nc.sync.nop(nofuse=True, hint="tail_waits")
            wait_clock.add_sem_waits(
                collector.ins,
                bass_rust.ScopedClock({None: tick_clock.global_clock}),
            )
            si = collector.ins.sync_info
            waits = list(si.on_wait) if si is not None and si.on_wait else []
            if len(waits) > 1:
                si.on_wait = [waits[0]]
                from concourse import mybir as _mybir

                for w in waits[1:]:
                    n = self.nc.sync.nop(nofuse=True, hint="tail_waits")
                    n.ins.sync_info = _mybir.SyncInfo(on_wait=[w], on_update=[])
            self.nc.sync.drain()
            self.nc.all_engine_barrier()
            assert self.sems is not None
            popped = self.nc._tile_sem_poison_stack.pop()
            assert popped is self._sem_poison
            self.nc.clear_and_free_semaphores(
                list(self.sems.allocated().values())
            )
            self.nc.all_engine_barrier()

    return SplitDrainTileContext


def _build_nc(repeat=1):
    import concourse.bass as bass
    import concourse.tile as tile
    import concourse.mybir as mybir
    from concourse.ap import AP

    tile_context_cls = _make_tile_context_cls()
    f32 = mybir.dt.float32
    f16 = mybir.dt.float16
    nc = bass.Bass("TRN2", target_bir_lowering=False, debug=False)
    x = nc.dram_tensor("x", [HW, BC], f32, kind="ExternalInput")
    # wsb[xi, (i, yi, j)] = w[i, j, yi*128+xi]: the per-partition SBUF
    # image of the weights, prepared host-side; SWDGE casts it to fp16
    wsb_d = nc.dram_tensor("wsb", [128, 1152], f16, kind="ExternalInput")
    # y stored fp16 (sigmoid output in [0,1]: adds <= ~2.4e-4 abs error)
    # and upcast to f32 on the host -- halves the output DMA traffic
    y = nc.dram_tensor("y", [HW, BC], f16, kind="ExternalOutput")

    NCH = H // LR  # 16 x row-chunks
    NGR = H // G  # 16 weight groups
    TW = 130  # T used width: col c = xi + j, lhsT reads cols 1:129
    TWS = 131  # T stored stride (!=TW so strided APs never dim-merge)

    with tile_context_cls(nc) as tc:
        with (
            tc.tile_pool(name="cn", bufs=1) as cpool,
            tc.tile_pool(name="xp", bufs=4) as xpool,
            tc.tile_pool(name="tp", bufs=5) as tpool,
            tc.tile_pool(name="op", bufs=4) as opool,
            tc.tile_pool(name="ps", bufs=2, space="PSUM") as ppool,
        ):
            # one-time: weight image (cast f32 -> fp16 in the DMA),
            # one-hot diagonal masks
            wsb = cpool.tile([128, 1152], f16)
            nc.sync.dma_start(out=wsb[:], in_=wsb_d.ap())
            i16 = mybir.dt.int16
            ones = cpool.tile([128, TW], i16)
            nc.gpsimd.memset(ones[:], 1)
            onesf = cpool.tile([128, TW], f16)
            nc.gpsimd.memset(onesf[:], 1.0)
            masks = cpool.tile([128, 3, TW], i16)
            for j in range(1, 3):
                # D_j[xi, c] = 1 where c - xi - j == 0
                nc.gpsimd.affine_select(
                    masks[:, j, :], ones[:],
                    pattern=[[1, TW]], base=-j, channel_multiplier=-1,
                    compare_op=mybir.AluOpType.is_equal, fill=0,
                )
            # fp16 one-hot for j=0 (used multiplicatively: zero-fills too)
            mask0f = cpool.tile([128, TW], f16)
            nc.gpsimd.affine_select(
                mask0f[:], onesf[:],
                pattern=[[1, TW]], base=0, channel_multiplier=-1,
                compare_op=mybir.AluOpType.is_equal, fill=0.0,
            )

            xt = {}
            tt = {}

            def load_chunk(c):  # noqa: closure rebound per repeat
                if c in xt or c >= NCH:
                    return
                t = xpool.tile([128, LR, BC], f16, tag="xchunk")
                # x rows [c*LR*128, (c+1)*LR*128): the SWDGE (gpsimd) DMA
                # casts f32 -> fp16 inline, so the DMA engines only move
                # half the bytes; 2 MiB per DMA keeps the Pool-side
                # descriptor generation cheap
                src = AP(
                    x.ap().tensor,
                    c * LR * 128 * BC,
                    [[BC, 128], [128 * BC, LR], [1, BC]],
                )
                nc.gpsimd.dma_start(out=t[:], in_=src)
                xt[c] = t

            def load_group(g, half=None):
                if (g, half) in tt or g >= NGR:
                    return
                gh = G if half is None else G // 2
                goff = 0 if not half or half == "lo" else G // 2
                t = tpool.tile([128, 3, G, TWS], f16, tag="T")
                ta = t[:]
                wv = wsb[:]
                for i in range(3):
                    out_i = AP(ta.tensor,
                               ta.offset + i * G * TWS + goff * TWS,
                               [[3 * G * TWS, 128], [TWS, gh], [1, TW]])

                    def wb(j):
                        return AP(wv.tensor,
                                  wv.offset + i * 384
                                  + (g * G + goff) * 3 + j,
                                  [[1152, 128], [3, gh], [0, TW]])

                    # j=0 as a multiply by the fp16 one-hot: zero-fills the
                    # whole block and places the j=0 diagonal in one pass
                    # (on gpsimd to keep DVE free for the predicated passes)
                    m0 = mask0f[:]
                    m0b = AP(m0.tensor, m0.offset,
                             [[TW, 128], [0, gh], [1, TW]])
                    nc.gpsimd.tensor_tensor(
                        out_i, m0b, wb(0), mybir.AluOpType.mult
                    )
                    for j in range(1, 3):
                        ma = masks[:, j, :]
                        mb = AP(ma.tensor, ma.offset + j,
                                [[3 * TW, 128], [0, gh], [1, 128]])
                        out_ij = AP(out_i.tensor, out_i.offset + j,
                                    [[3 * G * TWS, 128], [TWS, gh],
                                     [1, 128]])
                        wbj = wb(j)
                        wbj = AP(wbj.tensor, wbj.offset,
                                 [[1152, 128], [3, gh], [0, 128]])
                        nc.vector.copy_predicated(out_ij, mb, wbj)
                tt[(g, half)] = t

            rep_range = range(repeat)
            for _rep in rep_range:
              if _rep:
                xt.clear()
                tt.clear()
              # prime the pipeline: interleave the first chunk with the
              # first T groups so the opening matmul isn't queued behind
              # all the SWDGE descriptor-generation bursts on Pool
              load_chunk(0)
              load_group(0, "lo")
              load_group(0, "hi")
              load_group(1)
              load_chunk(1)
              load_group(2)
              load_chunk(2)
              for _g in range(3, 5):
                  load_group(_g)

              pt = None
              ystage = None
              for yo in range(H):
                  # prefetch beyond what this row touches
                  load_chunk((yo + 1) // LR + 1)
                  g1 = (yo + 1) // G + 1
                  g2 = (yo + 1) // G + 2
                  if g1 == NGR - 1:
                      load_group(g1, "lo")
                      load_group(g1, "hi")
                  else:
                      load_group(g1)
                  if g2 == NGR - 1:
                      load_group(g2, "lo")
                      load_group(g2, "hi")
                  else:
                      load_group(g2)

                  if yo % R == 0:
                      # 4 banks: rows yo..yo+3 accumulate side by side so
                      # one sigmoid covers all 4
                      pt = ppool.tile([128, R, BC], f32, tag="psum")
                      ystage = opool.tile([128, R, BC], f16, tag="yst")
                  yis = [yi for yi in (yo - 1, yo, yo + 1) if 0 <= yi < H]
                  for k, yi in enumerate(yis):
                      i_dy = yo - yi + 1
                      gg = yi // G
                      if gg in (0, NGR - 1):
                          key = (gg, "lo" if (yi % G) < G // 2 else "hi")
                      else:
                          key = (gg, None)
                      lhsT = tt[key][:, i_dy, yi % G, 1 : 1 + 128]  # [128, 128]
                      rhs = xt[yi // LR][:, yi % LR, :]
                      nc.tensor.matmul(
                          pt[:, yo % R, :],
                          lhsT,
                          rhs,
                          start=(k == 0),
                          stop=(k == len(yis) - 1),
                      )

                  if yo % R == R - 1:
                      nc.scalar.activation(
                          ystage[:],
                          pt[:],
                          mybir.ActivationFunctionType.Sigmoid,
                      )
                      c = yo // R
                      dst = AP(
                          y.ap().tensor,
                          c * R * 128 * BC,
                          [[BC, 128], [128 * BC, R], [1, BC]],
                      )
                      # stores issue from SP: its HWDGE ring is otherwise
                      # idle, keeping ACT free for the sigmoids
                      nc.sync.dma_start(out=dst, in_=ystage[:])
    return nc


def get_nc():
    if "nc" not in _CACHE:
        _CACHE["nc"] = _build_nc()
    return _CACHE["nc"]


def host_inputs(x: np.ndarray, w: np.ndarray):
    """Per-core input maps for the bass kernel (shared with test harness)."""
    x = np.ascontiguousarray(x, dtype=np.float32)
    wsb = np.ascontiguousarray(
        np.asarray(w, dtype=np.float32)
        .reshape(3, 3, H, W)
        .transpose(3, 0, 2, 1)
        .reshape(128, 1152)
        .astype(np.float16)
    )
    return [
        {"x": x[:, i * BC : (i + 1) * BC], "wsb": wsb} for i in range(NCORES)
    ]


def kernel(x: np.ndarray, w: np.ndarray) -> np.ndarray:
    import time as _time

    from concourse.bass_utils import run_bass_kernel_spmd

    nc = get_nc()
    in_maps = host_inputs(x, w)
    # The compile hook / remote execution path occasionally fails
    # transiently (observed: a flaky walrus invocation and a recoverable
    # NRT exec error); retry a few times before giving up.
    last_exc = None
    for attempt in range(4):
        try:
            res = run_bass_kernel_spmd(
                nc, in_maps, list(range(NCORES))
            ).results
            break
        except Exception as exc:  # noqa: BLE001
            last_exc = exc
            _time.sleep(2.0 * (attempt + 1))
    else:
        raise last_exc
    return np.ascontiguousarray(
        np.concatenate([res[i]["y"] for i in range(NCORES)], axis=1),
        dtype=np.float32,
    )


# revision 20
# speedup vs baseline: 1.5946x; 1.1563x over previous
"""Trainium2 Bass kernel for BCNLayer (3x3 per-position-weighted spatial
shift conv over a 128x128 grid + sigmoid).

y[yo,xo,b] = sigmoid( sum_{dy,dx in {-1,0,1}} w[dy+1,dx+1,(yo-dy)*128+(xo-dx)]
                      * x[(yo-dy)*128+(xo-dx), b] )   (zero outside the grid)

Formulation: for each output row yo, y_row[yo] = sigmoid( sum_{yi in
{yo-1,yo,yo+1}} T[dy,yi].T @ x_row[yi] ) where T[dy,yi] is a 128x128
tridiagonal matrix holding the three dx weight vectors of input row yi on
its diagonals (dy = yo-yi).  The whole pipeline runs in fp16 (x, w and y
all round-trip through fp16, ~1e-3 absolute on a sigmoid output -- far
inside the 2e-2 gate):

  * x chunks load through SWDGE DMAs that cast f32 -> fp16 inline,
    halving the bytes the DMA engines move;
  * each tridiagonal T block [G rows x 130 cols] is built by a SINGLE
    tensor_tensor multiply on DVE: a constant band mask (one at
    c - xi in {0,1,2}) times a stride-1 window of a host-PRE-SHIFTED
    weight image wsb[xi, i*WSTRIDE + 3*yi + j + xi] = w[i, j, yi, xi].
    The per-partition +xi shift baked into the image makes the tap that
    the stride-1 read lands on at column c exactly w[i, c-xi, ...], so
    the mask multiply places all three diagonals and the zero background
    in one dense pass -- no copy_predicated, and with every operand fp16
    stride-1 the DVE runs it in 2x mode;
  * matmuls are fp16 x fp16 -> f32 PSUM (1 cycle/row moving);
  * output rows accumulate into a 4-bank PSUM tile so one sigmoid
    activation covers 4 rows; a Pool/DVE tensor_scalar pass requantizes
    the fp16 sigmoid to u8 (x255 + 0.5, decoded /255 on the host, error
    <= ~2e-3), and the idle SP sequencer stores the u8 rows -- a quarter
    of the f32 output bytes.

T blocks are 130 wide (col c = xi + j) and the matmul reads cols 1:129,
so the x-boundary masking falls out of the padding columns.  The first
and last groups build in two half-blocks to shorten the pipeline ramp
and tail.

Sharding: data-parallel over batch, 4096/8 = 512 columns per core.
"""

import numpy as np

H = 128
W = 128
HW = H * W
B = 4096
NCORES = 8
BC = B // NCORES  # 512 batch columns per core
G = 8  # yi rows per weight-group tile
R = 2  # y rows per PSUM tile / sigmoid / store DMA
LR = 8  # x rows per load DMA (8 * 128 part * 512 * 2B = 1 MiB fp16)
TW = 130  # T used width: col c = xi + j, lhsT reads cols 1:129
WSTRIDE = 3 * H + TW  # pre-shifted weight image stride per dy block

_CACHE = {}


def _make_tile_context_cls():
    import concourse.tile as tile
    import bass_rust

    class SplitDrainTileContext(tile.TileContext):
        """The walrus build in this container accepts at most one sem-wait
        per instruction; Tile freely emits several (e.g. a matmul waiting
        on both operand DMA lanes).  Split the extras onto single-wait
        nops emitted just before the instruction on the same engine."""

        def _add_instruction(self, inst):
            from concourse import mybir as _mybir

            si = inst.sync_info
            if si is not None and si.on_wait and len(si.on_wait) > 1:
                waits = list(si.on_wait)
                si.on_wait = [waits[-1]]
                for w in waits[:-1]:
                    nop = _mybir.InstNoOp(
                        name=self.nc.get_next_instruction_name(),
                        ins=[],
                        outs=[],
                    )
                    nop.engine = inst.engine
                    nop.sync_info = _mybir.SyncInfo(on_wait=[w], on_update=[])
                    super()._add_instruction(nop)
            super()._add_instruction(inst)

        def _drain_and_barrier(self, tick_clock, wait_clock):
            collector = self.nc.sync.nop(nofuse=True, hint="tail_waits")
            wait_clock.add_sem_waits(
                collector.ins,
                bass_rust.ScopedClock({None: tick_clock.global_clock}),
            )
            si = collector.ins.sync_info
            waits = list(si.on_wait) if si is not None and si.on_wait else []
            if len(waits) > 1:
                si.on_wait = [waits[0]]
                from concourse import mybir as _mybir

                for w in waits[1:]:
                    n = self.nc.sync.nop(nofuse=True, hint="tail_waits")
                    n.ins.sync_info = _mybir.SyncInfo(on_wait=[w], on_update=[])
            self.nc.sync.drain()
            self.nc.all_engine_barrier()
            assert self.sems is not None
            popped = self.nc._tile_sem_poison_stack.pop()
            assert popped is self._sem_poison
            self.nc.clear_and_free_semaphores(
                list(self.sems.allocated().values())
            )
            self.nc.all_engine_barrier()

    return SplitDrainTileContext


def _build_nc(repeat=1):
    import concourse.bass as bass
    import concourse.tile as tile
    import concourse.mybir as mybir
    from concourse.ap import AP

    tile_context_cls = _make_tile_context_cls()
    f32 = mybir.dt.float32
    f16 = mybir.dt.float16
    nc = bass.Bass("TRN2", target_bir_lowering=False, debug=False)
    NCH = H // LR  # x row-chunks
    NGR = H // G  # weight groups
    TWS = 131  # T stored stride (!=TW so strided APs never dim-merge)

    x = nc.dram_tensor("x", [HW, BC], f32, kind="ExternalInput")
    # pre-shifted weight image (fp16, built host-side):
    #   wsb[xi, i*WSTRIDE + 3*yi + j + xi] = w[i, j, yi*128 + xi]
    wsb_d = nc.dram_tensor("wsb", [128, 3 * WSTRIDE], f16,
                           kind="ExternalInput")
    # band mask: 1.0 where 0 <= c - xi <= 2 else 0
    bm_d = nc.dram_tensor("bmask", [128, TW], f16, kind="ExternalInput")
    # y stored u8 (sigmoid output in [0,1] scaled by 255: adds <= ~2e-3
    # abs error, decoded /255 on the host) -- quarters the output traffic
    u8 = mybir.dt.uint8
    y = nc.dram_tensor("y", [HW, BC], u8, kind="ExternalOutput")

    with tile_context_cls(nc) as tc:
        with (
            tc.tile_pool(name="cn", bufs=1) as cpool,
            tc.tile_pool(name="xp", bufs=4) as xpool,
            tc.tile_pool(name="tp", bufs=5) as tpool,
            tc.tile_pool(name="op", bufs=4) as opool,
            tc.tile_pool(name="o8", bufs=4) as o8pool,
            tc.tile_pool(name="ps", bufs=4, space="PSUM") as ppool,
        ):
            # one-time: pre-shifted weight image + band mask (SP HWDGE --
            # tiny, and keeps Pool free for the x-chunk descriptor gen)
            wsb = cpool.tile([128, 3 * WSTRIDE], f16)
            nc.sync.dma_start(out=wsb[:], in_=wsb_d.ap())
            bmask = cpool.tile([128, TW], f16)
            nc.sync.dma_start(out=bmask[:], in_=bm_d.ap())

            xt = {}
            tt = {}

            def load_chunk(c):  # noqa: closure rebound per repeat
                if c in xt or c >= NCH:
                    return
                t = xpool.tile([128, LR, BC], f16, tag="xchunk")
                # x rows [c*LR*128, (c+1)*LR*128): the SWDGE (gpsimd) DMA
                # casts f32 -> fp16 inline, so the DMA engines only move
                # half the bytes
                src = AP(
                    x.ap().tensor,
                    c * LR * 128 * BC,
                    [[BC, 128], [128 * BC, LR], [1, BC]],
                )
                nc.gpsimd.dma_start(out=t[:], in_=src)
                xt[c] = t

            def load_group(g, half=None):
                if (g, half) in tt or g >= NGR:
                    return
                gh = G if half is None else G // 2
                goff = 0 if not half or half == "lo" else G // 2
                t = tpool.tile([128, 3, G, TWS], f16, tag="T")
                ta = t[:]
                wv = wsb[:]
                bv = bmask[:]
                mb = AP(bv.tensor, bv.offset,
                        [[TW, 128], [0, gh], [1, TW]])
                for i in range(3):
                    out_i = AP(ta.tensor,
                               ta.offset + i * G * TWS + goff * TWS,
                               [[3 * G * TWS, 128], [TWS, gh], [1, TW]])
                    # stride-1 window of the pre-shifted image: at column
                    # c the window holds w[i, c-xi, yi, xi] wherever the
                    # band mask is one
                    win = AP(wv.tensor,
                             wv.offset + i * WSTRIDE + (g * G + goff) * 3,
                             [[3 * WSTRIDE, 128], [3, gh], [1, TW]])
                    nc.vector.tensor_tensor(
                        out_i, mb, win, mybir.AluOpType.mult
                    )
                tt[(g, half)] = t

            rep_range = range(repeat)
            for _rep in rep_range:
              if _rep:
                xt.clear()
                tt.clear()
              # prime the pipeline; group 0 builds lo-half first so the
              # opening matmuls start as early as possible
              load_chunk(0)
              load_group(0, "lo")
              load_group(0, "hi")
              load_group(1)
              load_chunk(1)
              load_group(2)
              load_chunk(2)
              for _g in range(3, 5):
                  load_group(_g)

              pt = None
              ystage = None
              for yo in range(H):
                  # prefetch beyond what this row touches
                  load_chunk((yo + 1) // LR + 1)
                  g1 = (yo + 1) // G + 1
                  g2 = (yo + 1) // G + 2
                  for gp in (g1, g2):
                      if gp == NGR - 1:
                          load_group(gp, "lo")
                          load_group(gp, "hi")
                      else:
                          load_group(gp)

                  if yo % R == 0:
                      # 4 banks: rows yo..yo+3 accumulate side by side so
                      # one sigmoid covers all 4
                      pt = ppool.tile([128, R, BC], f32, tag="psum")
                      ystage = opool.tile([128, R, BC], f16, tag="yst")
                      y8stage = o8pool.tile([128, R, BC], u8, tag="y8")
                  yis = [yi for yi in (yo - 1, yo, yo + 1) if 0 <= yi < H]
                  for k, yi in enumerate(yis):
                      i_dy = yo - yi + 1
                      gg = yi // G
                      if gg in (0, NGR - 1):
                          key = (gg, "lo" if (yi % G) < G // 2 else "hi")
                      else:
                          key = (gg, None)
                      lhsT = tt[key][:, i_dy, yi % G, 1 : 1 + 128]
                      rhs = xt[yi // LR][:, yi % LR, :]
                      nc.tensor.matmul(
                          pt[:, yo % R, :],
                          lhsT,
                          rhs,
                          start=(k == 0),
                          stop=(k == len(yis) - 1),
                      )

                  if yo % R == R - 1:
                      nc.scalar.activation(
                          ystage[:],
                          pt[:],
                          mybir.ActivationFunctionType.Sigmoid,
                      )
                      c = yo // R
                      # u8 quantization pass (x255 + 0.5): alternate Pool
                      # and DVE so neither becomes the bottleneck (Pool
                      # also runs SWDGE, DVE also builds T); ratio 20:12
                      eng = nc.gpsimd if (c % 8) < 5 else nc.vector
                      eng.tensor_scalar(
                          y8stage[:], ystage[:], 255.0, 0.5,
                          mybir.AluOpType.mult, mybir.AluOpType.add,
                      )
                      dst = AP(
                          y.ap().tensor,
                          c * R * 128 * BC,
                          [[BC, 128], [128 * BC, R], [1, BC]],
                      )
                      # stores issue from SP: its HWDGE ring is otherwise
                      # idle, keeping ACT free for the sigmoids
                      nc.sync.dma_start(out=dst, in_=y8stage[:])
    return nc


def host_inputs(x: np.ndarray, w: np.ndarray):
    """Per-core input maps for the bass kernel (shared with test harness)."""
    x = np.ascontiguousarray(x, dtype=np.float32)
    w16 = (
        np.asarray(w, dtype=np.float32).reshape(3, 3, H, W).astype(np.float16)
    )
    # pre-shifted image: wsb[xi, i*WSTRIDE + 3*yi + j + xi] = w16[i, j, yi, xi]
    wsb = np.zeros((128, 3 * WSTRIDE), np.float16)
    xi = np.arange(128)
    for i in range(3):
        for j in range(3):
            for yi in range(H):
                wsb[xi, i * WSTRIDE + 3 * yi + j + xi] = w16[i, j, yi, :]
    bmask = np.zeros((128, TW), np.float16)
    c = np.arange(TW)[None, :]
    d = c - xi[:, None]
    bmask[(d >= 0) & (d <= 2)] = 1.0
    bmask = np.ascontiguousarray(bmask)
    wsb = np.ascontiguousarray(wsb)
    return [
        {"x": x[:, i * BC : (i + 1) * BC], "wsb": wsb, "bmask": bmask}
        for i in range(NCORES)
    ]


def get_nc():
    if "nc" not in _CACHE:
        _CACHE["nc"] = _build_nc()
    return _CACHE["nc"]


def kernel(x: np.ndarray, w: np.ndarray) -> np.ndarray:
    import time as _time

    from concourse.bass_utils import run_bass_kernel_spmd

    nc = get_nc()
    in_maps = host_inputs(x, w)
    # The compile hook / remote execution path occasionally fails
    # transiently; retry a few times before giving up.
    last_exc = None
    for attempt in range(4):
        try:
            res = run_bass_kernel_spmd(
                nc, in_maps, list(range(NCORES))
            ).results
            break
        except Exception as exc:  # noqa: BLE001
            last_exc = exc
            _time.sleep(2.0 * (attempt + 1))
    else:
        raise last_exc
    out = np.concatenate([res[i]["y"] for i in range(NCORES)], axis=1)
    return np.ascontiguousarray(out.astype(np.float32) / 255.0)


# revision 21
# speedup vs baseline: 3.5231x; 2.2094x over previous
"""Trainium2 Bass kernel for BCNLayer (3x3 per-position-weighted spatial
shift conv over a 128x128 grid + sigmoid).

y[yo,xo,b] = sigmoid( sum_{dy,dx in {-1,0,1}} w[dy+1,dx+1,(yo-dy)*128+(xo-dx)]
                      * x[(yo-dy)*128+(xo-dx), b] )   (zero outside the grid)

Formulation: for each output row yo, y_row[yo] = sigmoid( sum_{yi in
{yo-1,yo,yo+1}} T[dy,yi].T @ x_row[yi] ) where T[dy,yi] is a 128x128
tridiagonal matrix holding the three dx weight vectors of input row yi on
its diagonals (dy = yo-yi).  The whole pipeline runs in fp16 (x, w and y
all round-trip through fp16, ~1e-3 absolute on a sigmoid output -- far
inside the 2e-2 gate):

  * x chunks load through SWDGE DMAs that cast f32 -> fp16 inline,
    halving the bytes the DMA engines move;
  * each tridiagonal T block [G rows x 130 cols] is built by a SINGLE
    tensor_tensor multiply on DVE: a constant band mask (one at
    c - xi in {0,1,2}) times a stride-1 window of a host-PRE-SHIFTED
    weight image wsb[xi, i*WSTRIDE + 3*yi + j + xi] = w[i, j, yi, xi].
    The per-partition +xi shift baked into the image makes the tap that
    the stride-1 read lands on at column c exactly w[i, c-xi, ...], so
    the mask multiply places all three diagonals and the zero background
    in one dense pass -- no copy_predicated, and with every operand fp16
    stride-1 the DVE runs it in 2x mode;
  * matmuls are fp16 x fp16 -> f32 PSUM (1 cycle/row moving);
  * output rows accumulate into 2-row PSUM tiles so one sigmoid
    activation covers 2 rows, stored fp16 by the idle SP sequencer
    (DMA transfers overlap in flight, so loads, stores and the weight
    image all stream concurrently).

T blocks are 130 wide (col c = xi + j) and the matmul reads cols 1:129,
so the x-boundary masking falls out of the padding columns.  The first
and last groups build in two half-blocks to shorten the pipeline ramp
and tail.

Sharding: data-parallel over batch, 4096/8 = 512 columns per core.
"""

import numpy as np

H = 128
W = 128
HW = H * W
B = 4096
NCORES = 8
BC = B // NCORES  # 512 batch columns per core
G = 8  # yi rows per weight-group tile
R = 2  # y rows per PSUM tile / sigmoid / store DMA
LR = 8  # x rows per load DMA (8 * 128 part * 512 * 2B = 1 MiB fp16)
TW = 130  # T used width: col c = xi + j, lhsT reads cols 1:129
WSTRIDE = 3 * H + TW  # pre-shifted weight image stride per dy block

_CACHE = {}


def _make_tile_context_cls():
    import concourse.tile as tile
    import bass_rust

    class SplitDrainTileContext(tile.TileContext):
        """The walrus build in this container accepts at most one sem-wait
        per instruction; Tile freely emits several (e.g. a matmul waiting
        on both operand DMA lanes).  Split the extras onto single-wait
        nops emitted just before the instruction on the same engine."""

        def _add_instruction(self, inst):
            from concourse import mybir as _mybir

            si = inst.sync_info
            if si is not None and si.on_wait and len(si.on_wait) > 1:
                waits = list(si.on_wait)
                si.on_wait = [waits[-1]]
                for w in waits[:-1]:
                    nop = _mybir.InstNoOp(
                        name=self.nc.get_next_instruction_name(),
                        ins=[],
                        outs=[],
                    )
                    nop.engine = inst.engine
                    nop.sync_info = _mybir.SyncInfo(on_wait=[w], on_update=[])
                    super()._add_instruction(nop)
            super()._add_instruction(inst)

        def _drain_and_barrier(self, tick_clock, wait_clock):
            collector = self.nc.sync.nop(nofuse=True, hint="tail_waits")
            wait_clock.add_sem_waits(
                collector.ins,
                bass_rust.ScopedClock({None: tick_clock.global_clock}),
            )
            si = collector.ins.sync_info
            waits = list(si.on_wait) if si is not None and si.on_wait else []
            if len(waits) > 1:
                si.on_wait = [waits[0]]
                from concourse import mybir as _mybir

                for w in waits[1:]:
                    n = self.nc.sync.nop(nofuse=True, hint="tail_waits")
                    n.ins.sync_info = _mybir.SyncInfo(on_wait=[w], on_update=[])
            self.nc.sync.drain()
            self.nc.all_engine_barrier()
            assert self.sems is not None
            popped = self.nc._tile_sem_poison_stack.pop()
            assert popped is self._sem_poison
            self.nc.clear_and_free_semaphores(
                list(self.sems.allocated().values())
            )
            self.nc.all_engine_barrier()

    return SplitDrainTileContext


def _build_nc(repeat=1):
    import concourse.bass as bass
    import concourse.tile as tile
    import concourse.mybir as mybir
    from concourse.ap import AP

    tile_context_cls = _make_tile_context_cls()
    f32 = mybir.dt.float32
    f16 = mybir.dt.float16
    nc = bass.Bass("TRN2", target_bir_lowering=False, debug=False)
    NCH = H // LR  # x row-chunks
    NGR = H // G  # weight groups
    TWS = 131  # T stored stride (!=TW so strided APs never dim-merge)

    x = nc.dram_tensor("x", [HW, BC], f32, kind="ExternalInput")
    # pre-shifted weight image (fp16, built host-side):
    #   wsb[xi, i*WSTRIDE + 3*yi + j + xi] = w[i, j, yi*128 + xi]
    wsb_d = nc.dram_tensor("wsb", [128, 3 * WSTRIDE], f16,
                           kind="ExternalInput")
    # band mask: 1.0 where 0 <= c - xi <= 2 else 0
    bm_d = nc.dram_tensor("bmask", [128, TW], f16, kind="ExternalInput")
    # y stored fp16 (sigmoid output in [0,1]: adds <= ~2.4e-4 abs error)
    # and upcast to f32 on the host
    y = nc.dram_tensor("y", [HW, BC], f16, kind="ExternalOutput")

    with tile_context_cls(nc) as tc:
        with (
            tc.tile_pool(name="cn", bufs=1) as cpool,
            tc.tile_pool(name="xp", bufs=4) as xpool,
            tc.tile_pool(name="tp", bufs=5) as tpool,
            tc.tile_pool(name="op", bufs=4) as opool,
            tc.tile_pool(name="ps", bufs=3, space="PSUM") as ppool,
            tc.tile_pool(name="p1", bufs=2, space="PSUM") as p1pool,
        ):
            # one-time: band mask (ACT ring) + the pre-shifted weight
            # image as three per-dy strips on alternating HWDGE issuers --
            # DMA transfers overlap, so the strips land ~3x sooner than
            # one monolithic load and the first T build starts earlier
            bmask = cpool.tile([128, TW], f16)
            nc.scalar.dma_start(out=bmask[:], in_=bm_d.ap())
            wsb = cpool.tile([128, 3 * WSTRIDE], f16)
            for i in range(3):
                eng = nc.sync if i != 1 else nc.scalar
                srcw = AP(wsb_d.ap().tensor, i * WSTRIDE,
                          [[3 * WSTRIDE, 128], [1, WSTRIDE]])
                dstw = AP(wsb[:].tensor, wsb[:].offset + i * WSTRIDE,
                          [[3 * WSTRIDE, 128], [1, WSTRIDE]])
                eng.dma_start(out=dstw, in_=srcw)

            xt = {}
            tt = {}

            def load_chunk(c, split_first=False):  # noqa: rebound per rep
                if c in xt or c >= NCH:
                    return
                t = xpool.tile([128, LR, BC], f16, tag="xchunk")
                # x rows [c*LR*128, (c+1)*LR*128): the SWDGE (gpsimd) DMA
                # casts f32 -> fp16 inline, so the DMA engines only move
                # half the bytes

                def rows(lo, n):
                    srcr = AP(
                        x.ap().tensor,
                        (c * LR + lo) * 128 * BC,
                        [[BC, 128], [128 * BC, n], [1, BC]],
                    )
                    nc.gpsimd.dma_start(out=t[:, lo : lo + n, :], in_=srcr)

                if split_first:
                    rows(0, 2)
                    rows(2, LR - 2)
                else:
                    rows(0, LR)
                xt[c] = t

            def load_group(g, half=None):
                if (g, half) in tt or g >= NGR:
                    return
                gh = G if half is None else G // 2
                goff = 0 if not half or half == "lo" else G // 2
                t = tpool.tile([128, 3, G, TWS], f16, tag="T")
                ta = t[:]
                wv = wsb[:]
                bv = bmask[:]
                mb = AP(bv.tensor, bv.offset,
                        [[TW, 128], [0, gh], [1, TW]])
                for i in range(3):
                    out_i = AP(ta.tensor,
                               ta.offset + i * G * TWS + goff * TWS,
                               [[3 * G * TWS, 128], [TWS, gh], [1, TW]])
                    # stride-1 window of the pre-shifted image: at column
                    # c the window holds w[i, c-xi, yi, xi] wherever the
                    # band mask is one
                    win = AP(wv.tensor,
                             wv.offset + i * WSTRIDE + (g * G + goff) * 3,
                             [[3 * WSTRIDE, 128], [3, gh], [1, TW]])
                    nc.vector.tensor_tensor(
                        out_i, mb, win, mybir.AluOpType.mult
                    )
                tt[(g, half)] = t

            rep_range = range(repeat)
            for _rep in rep_range:
              if _rep:
                xt.clear()
                tt.clear()
              # prime the pipeline; group 0 builds lo-half first so the
              # opening matmuls start as early as possible
              load_chunk(0, split_first=True)
              load_group(0, "lo")
              load_group(0, "hi")
              load_group(1)
              load_chunk(1)
              load_group(2)
              load_chunk(2)
              for _g in range(3, 5):
                  load_group(_g)

              pt = None
              ystage = None
              for yo in range(H):
                  # prefetch beyond what this row touches
                  load_chunk((yo + 1) // LR + 1)
                  g1 = (yo + 1) // G + 1
                  g2 = (yo + 1) // G + 2
                  for gp in (g1, g2):
                      if gp == NGR - 1:
                          load_group(gp, "lo")
                          load_group(gp, "hi")
                      else:
                          load_group(gp)

                  if yo >= H - R:
                      # the final rows get 1-bank PSUM tiles so each
                      # sigmoid fires as soon as its own matmuls finish
                      pt1 = p1pool.tile([128, BC], f32, tag="psum1")
                      if yo % R == 0:
                          ystage = opool.tile([128, R, BC], f16, tag="yst")
                  elif yo % R == 0:
                      # banks: rows yo..yo+R-1 accumulate side by side so
                      # one sigmoid covers all R
                      pt = ppool.tile([128, R, BC], f32, tag="psum")
                      ystage = opool.tile([128, R, BC], f16, tag="yst")
                  pdst = pt1[:] if yo >= H - R else pt[:, yo % R, :]
                  yis = [yi for yi in (yo - 1, yo, yo + 1) if 0 <= yi < H]
                  for k, yi in enumerate(yis):
                      i_dy = yo - yi + 1
                      gg = yi // G
                      if gg in (0, NGR - 1):
                          key = (gg, "lo" if (yi % G) < G // 2 else "hi")
                      else:
                          key = (gg, None)
                      lhsT = tt[key][:, i_dy, yi % G, 1 : 1 + 128]
                      rhs = xt[yi // LR][:, yi % LR, :]
                      nc.tensor.matmul(
                          pdst,
                          lhsT,
                          rhs,
                          start=(k == 0),
                          stop=(k == len(yis) - 1),
                      )

                  if yo >= H - R:
                      # final rows: per-row sigmoid + store the moment the
                      # row's accumulation lands, on alternating rings
                      h = yo % R
                      nc.scalar.activation(
                          ystage[:, h, :],
                          pt1[:],
                          mybir.ActivationFunctionType.Sigmoid,
                      )
                      dst = AP(
                          y.ap().tensor,
                          yo * 128 * BC,
                          [[BC, 128], [1, BC]],
                      )
                      eng = nc.sync if h == 0 else nc.scalar
                      eng.dma_start(out=dst, in_=ystage[:, h, :])
                  elif yo % R == R - 1:
                      nc.scalar.activation(
                          ystage[:],
                          pt[:],
                          mybir.ActivationFunctionType.Sigmoid,
                      )
                      c = yo // R
                      dst = AP(
                          y.ap().tensor,
                          c * R * 128 * BC,
                          [[BC, 128], [128 * BC, R], [1, BC]],
                      )
                      # stores issue from SP: its HWDGE ring is otherwise
                      # idle, keeping ACT free for the sigmoids
                      nc.sync.dma_start(out=dst, in_=ystage[:])
    return nc


def host_inputs(x: np.ndarray, w: np.ndarray):
    """Per-core input maps for the bass kernel (shared with test harness)."""
    x = np.ascontiguousarray(x, dtype=np.float32)
    w16 = (
        np.asarray(w, dtype=np.float32).reshape(3, 3, H, W).astype(np.float16)
    )
    # pre-shifted image: wsb[xi, i*WSTRIDE + 3*yi + j + xi] = w16[i, j, yi, xi]
    wsb = np.zeros((128, 3 * WSTRIDE), np.float16)
    xi = np.arange(128)
    for i in range(3):
        for j in range(3):
            for yi in range(H):
                wsb[xi, i * WSTRIDE + 3 * yi + j + xi] = w16[i, j, yi, :]
    bmask = np.zeros((128, TW), np.float16)
    c = np.arange(TW)[None, :]
    d = c - xi[:, None]
    bmask[(d >= 0) & (d <= 2)] = 1.0
    bmask = np.ascontiguousarray(bmask)
    wsb = np.ascontiguousarray(wsb)
    return [
        {"x": x[:, i * BC : (i + 1) * BC], "wsb": wsb, "bmask": bmask}
        for i in range(NCORES)
    ]


def get_nc():
    if "nc" not in _CACHE:
        _CACHE["nc"] = _build_nc()
    return _CACHE["nc"]


def kernel(x: np.ndarray, w: np.ndarray) -> np.ndarray:
    import time as _time

    from concourse.bass_utils import run_bass_kernel_spmd

    nc = get_nc()
    in_maps = host_inputs(x, w)
    # The compile hook / remote execution path occasionally fails
    # transiently; retry a few times before giving up.
    last_exc = None
    for attempt in range(4):
        try:
            res = run_bass_kernel_spmd(
                nc, in_maps, list(range(NCORES))
            ).results
            break
        except Exception as exc:  # noqa: BLE001
            last_exc = exc
            _time.sleep(2.0 * (attempt + 1))
    else:
        raise last_exc
    out = np.concatenate([res[i]["y"] for i in range(NCORES)], axis=1)
    return np.ascontiguousarray(out.astype(np.float32))
